# revision 12
# baseline (speedup 1.0000x reference)
"""GwcVolume (group-wise correlation volume) Bass kernel for Trainium2.

Problem: left/right features [2, 320, 96, 312] fp32, GROUP=40, cpg=8,
max_disp=48.  Output cost volume [2, 40, 48, 96, 312]:
    cost[b,g,d,h,w] = mean_c( l[b,g,c,h,w] * r[b,g,c,h,w-d] ),  0 for w<d.

Strategy (8 NeuronCores):
  - Shard the 80 (b,g) pairs across cores, 10 per core.  Each pair is fully
    independent (no collectives).
  - TensorE does all multiply-accumulate work as block-diagonal matmuls:
    for each (bg, h-group of 16), SBUF holds l as [128 = 16h x 8c, W] and a
    host-prebuilt block-diagonal stationary image rs [128, 10*128] where
    the (unit, w'-block blk, h-quad q) stationary is
        rs[32q + 8hi + c, 128 blk + 32 hi + ww] = r[h, c, 32 blk + ww] / 0,
    h = 16 hg + 4 q + hi.  matmul (K=32 rows at strip 32q, M=128, N=79):
        out[(hi,ww), n] = sum_c r[h,c,w'0+ww] * l[h,c,w'0+n]
                        = cost[d=n-ww, h, w=w'0+n]  for 0 <= n-ww < 48.
    The 4 quads run on distinct PE row-strips and distinct PSUM banks,
    so they execute concurrently on the 32x32 sub-array grid.
  - VectorE/ScalarE evacuate PSUM into a w-major SBUF buffer, DMA'd to HBM
    densely.  The host does the final (free) rearrangement: band extraction
    (d = n - ww), zero triangle for w < d, and the layout transpose.

v2 over the fp32 baseline:
  - bf16 inputs and outputs (fp32 accumulate in PSUM); host casts back.
  - Partition-major HBM layouts ([128, UNITS, ...]) + one DMA per group of
    GRP_U units, so each DMA packet is GRP_U x (per-unit bytes) contiguous
    per partition: l 4.4KB, rs 15.4KB, out 37.9KB instead of 736B/2.5KB/6.3KB.
    This amortizes the ~30ns/packet DMA-engine overhead (16 engines/core).
"""

import os

import numpy as np

# --- geometry (hardcoded for this problem) ---
B, G, CPG, H, W = 2, 40, 8, 96, 312
D = 48                      # max_disp
N_CORES = 8
PAIRS = B * G               # 80 (b,g) pairs
BG_PER_CORE = PAIRS // N_CORES  # 10
HGROUPS = H // 16           # 6 groups of 16 h's
NBLK = 10                   # w'-blocks of 32 (covers w' in [0, 320))
MBLK = 32                   # w' per block
NW = MBLK + D - 1           # 79 moving columns per matmul
WL = 368                    # padded l width (312 + 56; max needed w = 366)
WR = 320                    # padded r width (312 + 8)
UNITS = BG_PER_CORE * HGROUPS   # 60 (bg, hgroup) units per core
RSW = NBLK * 128            # 1280 stationary-image cols per unit
GRP_U = 12                  # units per DMA group
NGRP = UNITS // GRP_U       # 10 groups
CHUNK_B = 5                 # w'-blocks per PSUM bank (5*79=395 <= 512 f32)

_NC_CACHE = {}


def _build_nc(dt_in_name="bfloat16", dt_out_name="bfloat16"):
    from concourse import bacc, mybir, tile
    import concourse.bass as bass  # noqa: F401

    dt_in = getattr(mybir.dt, dt_in_name)
    dt_out = getattr(mybir.dt, dt_out_name)
    f32 = mybir.dt.float32

    nc = bacc.Bacc("TRN2", target_bir_lowering=False, debug=False)
    l_dram = nc.dram_tensor("l", [128, UNITS, WL], dt_in, kind="ExternalInput")
    r_dram = nc.dram_tensor("rs", [128, UNITS, RSW], dt_in,
                            kind="ExternalInput")
    # out layout: blk = CHUNK_B*chunk + j  (chunk-major), one PSUM bank holds
    # CHUNK_B w'-blocks so evacuation is one big copy per (unit, chunk, q-set)
    o_dram = nc.dram_tensor(
        "o", [128, UNITS, 2, 4, CHUNK_B, NW], dt_out, kind="ExternalOutput")

    with tile.TileContext(nc) as tc:
        with (
            tc.tile_pool(name="lp", bufs=2) as lp,
            tc.tile_pool(name="rp", bufs=2) as rp,
            tc.tile_pool(name="evp", bufs=4) as evp,
            tc.tile_pool(name="psp", bufs=2, space="PSUM") as psp,
        ):
            for grp in range(NGRP):
                u0 = grp * GRP_U
                lt = lp.tile([128, GRP_U, WL], dt_in)
                rt = rp.tile([128, GRP_U, RSW], dt_in)
                nc.sync.dma_start(lt[:], l_dram[:, u0:u0 + GRP_U, :])
                nc.sync.dma_start(rt[:], r_dram[:, u0:u0 + GRP_U, :])
                for ui in range(GRP_U):
                    ev = evp.tile([128, 1, 2, 4, CHUNK_B, NW], dt_out)
                    for ch in range(2):
                        # one PSUM bank (512 f32) per quad holds CHUNK_B blks
                        ps = psp.tile([128, 4, 512], f32)
                        for j in range(CHUNK_B):
                            blk = CHUNK_B * ch + j
                            for q in range(4):
                                nc.tensor.matmul(
                                    out=ps[:, q, NW * j:NW * j + NW],
                                    lhsT=rt[32 * q:32 * q + 32, ui,
                                            128 * blk:128 * blk + 128],
                                    rhs=lt[32 * q:32 * q + 32, ui,
                                           MBLK * blk:MBLK * blk + NW],
                                    start=True,
                                    stop=True,
                                    tile_position=(32 * q, 0),
                                )
                        if ch == 0:
                            nc.vector.tensor_copy(
                                out=ev[:, 0, ch], in_=ps[:, :, 0:CHUNK_B * NW])
                        else:
                            nc.scalar.copy(
                                out=ev[:, 0, ch], in_=ps[:, :, 0:CHUNK_B * NW])
                    # store per unit from the idle gpsimd queue so the next
                    # group's loads don't queue behind it on sync
                    nc.gpsimd.dma_start(
                        o_dram[:, u0 + ui:u0 + ui + 1], ev[:])
    nc.compile()
    return nc


def _get_nc(key=("bfloat16", "bfloat16")):
    if key not in _NC_CACHE:
        _NC_CACHE[key] = _build_nc(*key)
    return _NC_CACHE[key]


def _np_dtype(name):
    if name == "bfloat16":
        import ml_dtypes
        return ml_dtypes.bfloat16
    return np.float32


def _pack_inputs(left, right, dt_np):
    """-> per-core in_maps; l pre-scaled by 1/cpg, r as block-diag image.

    HBM layouts are partition-major: l [128, UNITS, WL], rs [128, UNITS, RSW]
    per core, so group DMAs get large contiguous per-partition packets.
    """
    # [B, C, H, W] -> [B, G, cpg, H, W] -> [pair, H, cpg, W]
    l5 = left.reshape(B, G, CPG, H, W).transpose(0, 1, 3, 2, 4).reshape(
        PAIRS, H, CPG, W)
    r5 = right.reshape(B, G, CPG, H, W).transpose(0, 1, 3, 2, 4).reshape(
        PAIRS, H, CPG, W)
    lp = np.zeros((PAIRS, H, CPG, WL), dtype=np.float32)
    lp[..., :W] = l5 * (1.0 / CPG)
    lp = lp.astype(dt_np)
    # l: [pair, H=6*16, cpg, WL] -> per core [UNITS, 128, WL]
    lp = lp.reshape(N_CORES, UNITS, 128, WL)

    rp = np.zeros((PAIRS, H, CPG, WR), dtype=np.float32)
    rp[..., :W] = r5
    rp = rp.astype(dt_np)
    # block-diagonal stationary image:
    # axes: [pair, hg, q, hi_row, c, blk, hi_col, ww]
    rv = rp.reshape(PAIRS, HGROUPS, 4, 4, CPG, NBLK, MBLK)
    rb = np.zeros((PAIRS, HGROUPS, 4, 4, CPG, NBLK, 4, MBLK), dtype=dt_np)
    for i in range(4):
        rb[:, :, :, i, :, :, i, :] = rv[:, :, :, i, :, :, :]
    rb = rb.reshape(N_CORES, UNITS, 128, RSW)
    return [
        {"l": np.ascontiguousarray(lp[k].transpose(1, 0, 2)),
         "rs": np.ascontiguousarray(rb[k].transpose(1, 0, 2))}
        for k in range(N_CORES)
    ]


def _unpack_outputs(outs):
    """outs: 8 arrays [128, UNITS, 2, 4, CHUNK_B, NW] -> [B,G,D,H,W] fp32."""
    # -> [UNITS, 128, NBLK, 4, NW] with blk = CHUNK_B*ch + j
    O = np.stack(
        [np.asarray(o).astype(np.float32).transpose(1, 0, 2, 4, 3, 5)
         .reshape(UNITS, 128, NBLK, 4, NW)
         for o in outs])
    # [80pair, 6hg, 4hi, 32ww, 10blk, 4q, 79n]
    O = O.reshape(PAIRS, HGROUPS, 4, MBLK, NBLK, 4, NW)
    WPAD = 368
    final = np.zeros((PAIRS, D, H, WPAD), dtype=np.float32)
    s0, sd, sh, sw = (np.array(final.strides) // final.itemsize)
    st = np.lib.stride_tricks.as_strided
    it = final.itemsize
    for q in range(4):
        for hi in range(4):
            h0 = 4 * q + hi
            A = O[:, :, hi, :, :, q, :]  # [80, 6, 32ww, 10blk, 79n] view
            a = np.array(A.strides) // it
            V = st(A, shape=(PAIRS, HGROUPS, MBLK, NBLK, D),
                   strides=tuple(np.array([a[0], a[1], a[2] + a[4], a[3],
                                           a[4]]) * it))
            # dest: final[pair, d, 16*hg + h0, 32*blk + ww + d]
            Dv = st(final[:, :, h0:, :],
                    shape=(PAIRS, HGROUPS, MBLK, NBLK, D),
                    strides=tuple(np.array([s0, 16 * sh, sw, MBLK * sw,
                                            sd + sw]) * it))
            Dv[...] = V
    return final[:, :, :, :W].reshape(B, G, D, H, W)


def _install_profile_hook():
    """Make trace=True work when the image's antenv lacks axon_hooks."""
    import sys
    import types
    try:
        from antenv.axon_hooks import get_axon_ntff_profile_hook  # noqa: F401
        return
    except ImportError:
        pass
    if "/root/.axon_site" not in sys.path:
        sys.path.insert(0, "/root/.axon_site")
    from trn_agent_boot.trn_boot import _ntff_profile_via_ctypes
    hook = _ntff_profile_via_ctypes("/opt/axon/libaxon_pjrt.so")
    import antenv
    mod = types.ModuleType("antenv.axon_hooks")
    state = {"hook": hook}
    mod.get_axon_ntff_profile_hook = lambda: state["hook"]
    mod.set_axon_ntff_profile_hook = lambda h: state.update(hook=h)
    sys.modules["antenv.axon_hooks"] = mod
    antenv.axon_hooks = mod


def kernel(left_feature, right_feature, max_disp):
    import sys
    if "/opt/trn_rl_repo" not in sys.path:
        sys.path.insert(0, "/opt/trn_rl_repo")
    from concourse import bass_utils
    from concourse.bass_utils import run_bass_kernel_spmd

    left = np.asarray(left_feature, dtype=np.float32)
    right = np.asarray(right_feature, dtype=np.float32)
    assert int(max_disp) == D
    assert left.shape == (B, G * CPG, H, W)

    dt_in_name = os.environ.get("GWC_DT_IN", "bfloat16")
    dt_out_name = os.environ.get("GWC_DT_OUT", "bfloat16")
    dt_np = _np_dtype(dt_in_name)
    nc = _get_nc((dt_in_name, dt_out_name))
    in_maps = _pack_inputs(left, right, dt_np)

    trace = bool(os.environ.get("GWC_PROFILE"))
    if trace:
        _install_profile_hook()
        bass_utils.upload_artifacts = lambda tmpdir: str(tmpdir)  # no bucket
    res = run_bass_kernel_spmd(
        nc, in_maps, core_ids=list(range(N_CORES)), trace=trace
    )
    if trace:
        kernel._last_profile = res
        print(f"[kernel] exec_time_ns={res.exec_time_ns} "
              f"mean={res.mean_exec_time_ns}", flush=True)
    outs = [res.results[k]["o"] for k in range(N_CORES)]
    return _unpack_outputs(outs)


# revision 16
# speedup vs baseline: 1.0665x; 1.0665x over previous
"""GwcVolume (group-wise correlation volume) Bass kernel for Trainium2.

Problem: left/right features [2, 320, 96, 312] fp32, GROUP=40, cpg=8,
max_disp=48.  Output cost volume [2, 40, 48, 96, 312]:
    cost[b,g,d,h,w] = mean_c( l[b,g,c,h,w] * r[b,g,c,h,w-d] ),  0 for w<d.

Strategy (8 NeuronCores):
  - Shard the 80 (b,g) pairs across cores, 10 per core.  Each pair is fully
    independent (no collectives).
  - TensorE does all multiply-accumulate work as block-diagonal matmuls:
    for each (bg, h-group of 16), SBUF holds l as [128 = 16h x 8c, W] and a
    host-prebuilt block-diagonal stationary image rs [128, 10*128] where
    the (unit, w'-block blk, h-quad q) stationary is
        rs[32q + 8hi + c, 128 blk + 32 hi + ww] = r[h, c, 32 blk + ww] / 0,
    h = 16 hg + 4 q + hi.  matmul (K=32 rows at strip 32q, M=128, N=79):
        out[(hi,ww), n] = sum_c r[h,c,w'0+ww] * l[h,c,w'0+n]
                        = cost[d=n-ww, h, w=w'0+n]  for 0 <= n-ww < 48.
    The 4 quads run on distinct PE row-strips and distinct PSUM banks,
    so they execute concurrently on the 32x32 sub-array grid.
  - VectorE/ScalarE evacuate PSUM into a w-major SBUF buffer, DMA'd to HBM
    densely.  The host does the final (free) rearrangement: band extraction
    (d = n - ww), zero triangle for w < d, and the layout transpose.

v2 over the fp32 baseline:
  - bf16 inputs and outputs (fp32 accumulate in PSUM); host casts back.
  - Partition-major HBM layouts ([128, UNITS, ...]) + one DMA per group of
    GRP_U units, so each DMA packet is GRP_U x (per-unit bytes) contiguous
    per partition: l 4.4KB, rs 15.4KB, out 37.9KB instead of 736B/2.5KB/6.3KB.
    This amortizes the ~30ns/packet DMA-engine overhead (16 engines/core).
"""

import os

import numpy as np

# --- geometry (hardcoded for this problem) ---
B, G, CPG, H, W = 2, 40, 8, 96, 312
D = 48                      # max_disp
N_CORES = 8
PAIRS = B * G               # 80 (b,g) pairs
BG_PER_CORE = PAIRS // N_CORES  # 10
HGROUPS = H // 16           # 6 groups of 16 h's
NBLK = 10                   # w'-blocks of 32 (covers w' in [0, 320))
MBLK = 32                   # w' per block
NW = MBLK + D - 1           # 79 moving columns per matmul
WL = 368                    # padded l width (312 + 56; max needed w = 366)
WR = 320                    # padded r width (312 + 8)
UNITS = BG_PER_CORE * HGROUPS   # 60 (bg, hgroup) units per core
RSW = NBLK * 128            # 1280 stationary-image cols per unit
GRP_U = 6                   # units per DMA group
NGRP = UNITS // GRP_U       # 10 groups
CHUNK_B = 5                 # w'-blocks per PSUM bank (5*79=395 <= 512 f32)

_NC_CACHE = {}


def _build_nc(dt_in_name="bfloat16", dt_out_name="bfloat16"):
    from concourse import bacc, mybir, tile
    import concourse.bass as bass  # noqa: F401

    dt_in = getattr(mybir.dt, dt_in_name)
    dt_out = getattr(mybir.dt, dt_out_name)
    f32 = mybir.dt.float32

    nc = bacc.Bacc("TRN2", target_bir_lowering=False, debug=False)
    l_dram = nc.dram_tensor("l", [128, UNITS, WL], dt_in, kind="ExternalInput")
    r_dram = nc.dram_tensor("rs", [128, UNITS, RSW], dt_in,
                            kind="ExternalInput")
    # out layout: blk = CHUNK_B*chunk + j  (chunk-major), one PSUM bank holds
    # CHUNK_B w'-blocks so evacuation is one big copy per (unit, chunk, q-set)
    o_dram = nc.dram_tensor(
        "o", [128, UNITS, 2, 4, CHUNK_B, NW], dt_out, kind="ExternalOutput")

    with tile.TileContext(nc) as tc:
        with (
            tc.tile_pool(name="lp", bufs=3) as lp,
            tc.tile_pool(name="rp", bufs=3) as rp,
            tc.tile_pool(name="evp", bufs=2) as evp,
            tc.tile_pool(name="psp", bufs=2, space="PSUM") as psp,
        ):
            lt = None
            for grp in range(NGRP):
                u0 = grp * GRP_U
                if grp % 2 == 0:
                    # l is small: load 2 groups at once for 8.8KB packets
                    lt = lp.tile([128, 2 * GRP_U, WL], dt_in)
                    nc.sync.dma_start(
                        lt[:], l_dram[:, u0:u0 + 2 * GRP_U, :])
                lofs = (grp % 2) * GRP_U
                rt = rp.tile([128, GRP_U, RSW], dt_in)
                nc.sync.dma_start(rt[:], r_dram[:, u0:u0 + GRP_U, :])
                ev = evp.tile([128, GRP_U, 2, 4, CHUNK_B, NW], dt_out)
                for ui in range(GRP_U):
                    for ch in range(2):
                        # one PSUM bank (512 f32) per quad holds CHUNK_B blks
                        ps = psp.tile([128, 4, 512], f32)
                        for j in range(CHUNK_B):
                            blk = CHUNK_B * ch + j
                            for q in range(4):
                                nc.tensor.matmul(
                                    out=ps[:, q, NW * j:NW * j + NW],
                                    lhsT=rt[32 * q:32 * q + 32, ui,
                                            128 * blk:128 * blk + 128],
                                    rhs=lt[32 * q:32 * q + 32, lofs + ui,
                                           MBLK * blk:MBLK * blk + NW],
                                    start=True,
                                    stop=True,
                                    tile_position=(32 * q, 0),
                                )
                        if ch == 0:
                            nc.vector.tensor_copy(
                                out=ev[:, ui, ch], in_=ps[:, :, 0:CHUNK_B * NW])
                        else:
                            nc.scalar.copy(
                                out=ev[:, ui, ch], in_=ps[:, :, 0:CHUNK_B * NW])
                    # store per pair of units from the idle gpsimd queue so
                    # next group's loads don't queue behind it on sync;
                    # per single unit in the last group for a shorter drain
                    if grp == NGRP - 1:
                        nc.gpsimd.dma_start(
                            o_dram[:, u0 + ui:u0 + ui + 1], ev[:, ui:ui + 1])
                    elif ui % 2 == 1:
                        nc.gpsimd.dma_start(
                            o_dram[:, u0 + ui - 1:u0 + ui + 1],
                            ev[:, ui - 1:ui + 1])
    nc.compile()
    return nc


def _get_nc(key=("bfloat16", "bfloat16")):
    if key not in _NC_CACHE:
        _NC_CACHE[key] = _build_nc(*key)
    return _NC_CACHE[key]


def _np_dtype(name):
    if name == "bfloat16":
        import ml_dtypes
        return ml_dtypes.bfloat16
    return np.float32


def _pack_inputs(left, right, dt_np):
    """-> per-core in_maps; l pre-scaled by 1/cpg, r as block-diag image.

    HBM layouts are partition-major: l [128, UNITS, WL], rs [128, UNITS, RSW]
    per core, so group DMAs get large contiguous per-partition packets.
    """
    # [B, C, H, W] -> [B, G, cpg, H, W] -> [pair, H, cpg, W]
    l5 = left.reshape(B, G, CPG, H, W).transpose(0, 1, 3, 2, 4).reshape(
        PAIRS, H, CPG, W)
    r5 = right.reshape(B, G, CPG, H, W).transpose(0, 1, 3, 2, 4).reshape(
        PAIRS, H, CPG, W)
    lp = np.zeros((PAIRS, H, CPG, WL), dtype=np.float32)
    lp[..., :W] = l5 * (1.0 / CPG)
    lp = lp.astype(dt_np)
    # l: [pair, H=6*16, cpg, WL] -> per core [UNITS, 128, WL]
    lp = lp.reshape(N_CORES, UNITS, 128, WL)

    rp = np.zeros((PAIRS, H, CPG, WR), dtype=np.float32)
    rp[..., :W] = r5
    rp = rp.astype(dt_np)
    # block-diagonal stationary image:
    # axes: [pair, hg, q, hi_row, c, blk, hi_col, ww]
    rv = rp.reshape(PAIRS, HGROUPS, 4, 4, CPG, NBLK, MBLK)
    rb = np.zeros((PAIRS, HGROUPS, 4, 4, CPG, NBLK, 4, MBLK), dtype=dt_np)
    for i in range(4):
        rb[:, :, :, i, :, :, i, :] = rv[:, :, :, i, :, :, :]
    rb = rb.reshape(N_CORES, UNITS, 128, RSW)
    return [
        {"l": np.ascontiguousarray(lp[k].transpose(1, 0, 2)),
         "rs": np.ascontiguousarray(rb[k].transpose(1, 0, 2))}
        for k in range(N_CORES)
    ]


def _unpack_outputs(outs):
    """outs: 8 arrays [128, UNITS, 2, 4, CHUNK_B, NW] -> [B,G,D,H,W] fp32."""
    # -> [UNITS, 128, NBLK, 4, NW] with blk = CHUNK_B*ch + j
    O = np.stack(
        [np.asarray(o).astype(np.float32).transpose(1, 0, 2, 4, 3, 5)
         .reshape(UNITS, 128, NBLK, 4, NW)
         for o in outs])
    # [80pair, 6hg, 4hi, 32ww, 10blk, 4q, 79n]
    O = O.reshape(PAIRS, HGROUPS, 4, MBLK, NBLK, 4, NW)
    WPAD = 368
    final = np.zeros((PAIRS, D, H, WPAD), dtype=np.float32)
    s0, sd, sh, sw = (np.array(final.strides) // final.itemsize)
    st = np.lib.stride_tricks.as_strided
    it = final.itemsize
    for q in range(4):
        for hi in range(4):
            h0 = 4 * q + hi
            A = O[:, :, hi, :, :, q, :]  # [80, 6, 32ww, 10blk, 79n] view
            a = np.array(A.strides) // it
            V = st(A, shape=(PAIRS, HGROUPS, MBLK, NBLK, D),
                   strides=tuple(np.array([a[0], a[1], a[2] + a[4], a[3],
                                           a[4]]) * it))
            # dest: final[pair, d, 16*hg + h0, 32*blk + ww + d]
            Dv = st(final[:, :, h0:, :],
                    shape=(PAIRS, HGROUPS, MBLK, NBLK, D),
                    strides=tuple(np.array([s0, 16 * sh, sw, MBLK * sw,
                                            sd + sw]) * it))
            Dv[...] = V
    return final[:, :, :, :W].reshape(B, G, D, H, W)


def _install_profile_hook():
    """Make trace=True work when the image's antenv lacks axon_hooks."""
    import sys
    import types
    try:
        from antenv.axon_hooks import get_axon_ntff_profile_hook  # noqa: F401
        return
    except ImportError:
        pass
    if "/root/.axon_site" not in sys.path:
        sys.path.insert(0, "/root/.axon_site")
    from trn_agent_boot.trn_boot import _ntff_profile_via_ctypes
    hook = _ntff_profile_via_ctypes("/opt/axon/libaxon_pjrt.so")
    import antenv
    mod = types.ModuleType("antenv.axon_hooks")
    state = {"hook": hook}
    mod.get_axon_ntff_profile_hook = lambda: state["hook"]
    mod.set_axon_ntff_profile_hook = lambda h: state.update(hook=h)
    sys.modules["antenv.axon_hooks"] = mod
    antenv.axon_hooks = mod


def kernel(left_feature, right_feature, max_disp):
    import sys
    if "/opt/trn_rl_repo" not in sys.path:
        sys.path.insert(0, "/opt/trn_rl_repo")
    from concourse import bass_utils
    from concourse.bass_utils import run_bass_kernel_spmd

    left = np.asarray(left_feature, dtype=np.float32)
    right = np.asarray(right_feature, dtype=np.float32)
    assert int(max_disp) == D
    assert left.shape == (B, G * CPG, H, W)

    dt_in_name = os.environ.get("GWC_DT_IN", "bfloat16")
    dt_out_name = os.environ.get("GWC_DT_OUT", "bfloat16")
    dt_np = _np_dtype(dt_in_name)
    nc = _get_nc((dt_in_name, dt_out_name))
    in_maps = _pack_inputs(left, right, dt_np)

    trace = bool(os.environ.get("GWC_PROFILE"))
    if trace:
        _install_profile_hook()
        bass_utils.upload_artifacts = lambda tmpdir: str(tmpdir)  # no bucket
    res = run_bass_kernel_spmd(
        nc, in_maps, core_ids=list(range(N_CORES)), trace=trace
    )
    if trace:
        kernel._last_profile = res
        print(f"[kernel] exec_time_ns={res.exec_time_ns} "
              f"mean={res.mean_exec_time_ns}", flush=True)
    outs = [res.results[k]["o"] for k in range(N_CORES)]
    return _unpack_outputs(outs)
